# revision 13
# baseline (speedup 1.0000x reference)
"""Aitchison multi-head attention on 8 trn2 NeuronCores.

Strategy:
- CLR centering is linear -> folded into Wq/Wk + biases on the host (fp64).
- Shard: core c handles batch b=c//4 and 4 heads (feature slice of 256).
  QKV/out projection weights sliced per core; host sums the 4 partial
  output projections per batch and adds bo.
- Device kernel (per core, bf16 matmul operands / fp32 PSUM accum):
    qcT,kcT [256,2048] = W_eff @ x.T (+bias, f on partitions)
    v_pl    [2048, 4x65] = x @ Wv.T + bv with a ones column appended per
            head: the PV matmul (lhsT=[tk,65]) then produces the softmax
            denominator as PSUM row 64 for free -- no separate ones-lhs
            sums matmuls (those were 25% of all PE streaming cycles).
    Per unit (head-pair p, 512-wide q block): 8 score groups, each
    [128,1024] PSUM (1 tk tile x 2 heads) x2 double-buffered; one
    2048-wide exp per group into a per-group [128,2048] bf16 tile
    (bufs=4 rolling window keeps SBUF small).
    PV runs IN-unit, one group behind the exp (chunk g-1 after scores of
    group g), into two per-head [65,512] PSUM banks; banks are copied to
    SBUF right after chunk 7 so the next unit's start=True chain reuses
    them after a ~1-group handoff.
    1/Z via reciprocal_approx_fast (5x plain reciprocal), broadcast on
    GPSIMD, normalize mult on DVE into attnT (bf16).
    out partial = attnT.T @ WoT per 128x512 tile -> DVE copy -> DMA out.
- Emission is software-pipelined: the Q/K/V projections beyond the
  lead-in (kcT[0] + first qcT block) are interleaved into early units as
  PE filler; Wo groups drain during the last units and the tail.
"""
import sys
import types

sys.path.insert(0, "/opt/trn_rl_repo")

import numpy as np
import ml_dtypes

import concourse.bass as bass
import concourse.tile as tile
from concourse import bacc, mybir
from concourse.bass_utils import run_bass_kernel_spmd

B, T, E, H, Dh = 2, 2048, 1024, 16, 64
NCORES = 8
HPC = 4            # heads per core
F = HPC * Dh       # 256 features per core
SCALE = 8.0        # sqrt(Dh)
KC = E // 128      # 8 k-chunks in projections
BF = mybir.dt.bfloat16
F32 = mybir.dt.float32
BF_NP = ml_dtypes.bfloat16


def _install_ntff_hook():
    """trace=True under axon needs antenv.axon_hooks, missing in this image."""
    if "antenv.axon_hooks" in sys.modules:
        return
    try:
        from trn_agent_boot.trn_boot import _ntff_profile_via_ctypes

        hook = _ntff_profile_via_ctypes("/opt/axon/libaxon_pjrt.so")
    except Exception:
        hook = None
    mod = types.ModuleType("antenv.axon_hooks")
    mod.get_axon_ntff_profile_hook = lambda: hook
    sys.modules["antenv.axon_hooks"] = mod


def _emit(tc, io):
    nc = tc.nc
    from contextlib import ExitStack

    ctx = ExitStack()
    with ctx:
        const = ctx.enter_context(tc.tile_pool(name="const", bufs=1))
        xpool = ctx.enter_context(tc.tile_pool(name="x", bufs=24))
        qk = ctx.enter_context(tc.tile_pool(name="qk", bufs=1))
        epool = ctx.enter_context(tc.tile_pool(name="exp", bufs=4))
        spool = ctx.enter_context(tc.tile_pool(name="small", bufs=2))
        opool = ctx.enter_context(tc.tile_pool(name="out", bufs=2))
        ps_a = ctx.enter_context(tc.tile_pool(name="psa", bufs=2, space="PSUM"))
        ps_pv = ctx.enter_context(tc.tile_pool(name="pspv", bufs=1, space="PSUM"))
        ps_b = ctx.enter_context(tc.tile_pool(name="psb", bufs=2, space="PSUM"))

        def load_w(name):
            ts = []
            for kk in range(KC):
                t = const.tile([128, F], BF, name=f"{name}{kk}", tag=f"{name}{kk}")
                nc.sync.dma_start(t[:], io[name][kk * 128:(kk + 1) * 128, :])
                ts.append(t)
            return ts

        def load_b(name):
            ts = []
            for ft in range(2):
                t = const.tile([128, 1], F32, name=f"{name}{ft}", tag=f"{name}{ft}")
                nc.sync.dma_start(t[:], io[name][ft * 128:(ft + 1) * 128, :])
                ts.append(t)
            return ts

        def load_x(which):
            xc = []
            for kk in range(KC):
                t = xpool.tile([128, T], BF, name="xc", tag="xc")
                nc.sync.dma_start(t[:], io[which][kk * 128:(kk + 1) * 128, :])
                xc.append(t)
            return xc

        # ---- persistent activation tiles ----
        qcT = [qk.tile([128, T], BF, name=f"qcT{ft}", tag=f"qcT{ft}") for ft in range(2)]
        kcT = [qk.tile([128, T], BF, name=f"kcT{ft}", tag=f"kcT{ft}") for ft in range(2)]
        attnT = [qk.tile([128, T], BF, name=f"attnT{ft}", tag=f"attnT{ft}") for ft in range(2)]
        # v with a ones column per head: [t, 4 heads x (64 v | 1 one)]
        v_pl = [qk.tile([128, HPC, Dh + 1], BF, name=f"vpl{tt}", tag=f"vpl{tt}")
                for tt in range(16)]

        def proj_qk(wt, bt, dst, xc, ft, tbps=(0, 1, 2, 3)):
            for tbp in tbps:  # 512-wide t groups (psb rotation, no stalls)
                ps = ps_b.tile([128, 512], F32, name="psp", tag="psb")
                tq0 = tbp * 512
                for kk in range(KC):
                    nc.tensor.matmul(
                        ps[:],
                        wt[kk][:, ft * 128:(ft + 1) * 128],
                        xc[kk][:, tq0:tq0 + 512],
                        start=(kk == 0),
                        stop=(kk == KC - 1),
                    )
                nc.vector.tensor_scalar_add(
                    dst[ft][:, tq0:tq0 + 512], ps[:], bt[ft][:]
                )

        def v_tile(xc, wv_t, bv_bc, tt):
            ps = ps_b.tile([128, 256], F32, name="psv", tag="psb")
            for kk in range(KC):
                nc.tensor.matmul(
                    ps[:],
                    xc[kk][:, tt * 128:(tt + 1) * 128],
                    wv_t[kk][:],
                    start=(kk == 0),
                    stop=(kk == KC - 1),
                )
            # bias-add into the per-head 64-wide blocks (ones col untouched)
            nc.vector.tensor_tensor(
                v_pl[tt][:, :, 0:Dh],
                ps[:].rearrange("p (h d) -> p h d", h=HPC),
                bv_bc[:, :, :],
                mybir.AluOpType.add,
            )
            nc.gpsimd.memset(v_pl[tt][:, :, Dh:Dh + 1], 1.0)

        # --- PV: per-head [65,512] PSUM banks; row 64 = softmax sum (ones
        # column of v). start=True on the first chunk owns the bank.
        def pv_chunk(pvs, p, etile, g):
            for hh in range(2):
                lh = p * 2 + hh
                sl = etile[:, hh * 512:(hh + 1) * 512]
                nc.tensor.matmul(
                    pvs[hh][:],
                    v_pl[g][:, lh, :],
                    sl,
                    start=(g == 0),
                    stop=(g == 15),
                    skip_group_check=True,
                )

        def pv_copy(pvs):
            pvcs = []
            for hh in range(2):
                pvc = spool.tile([65, 512], F32, name=f"pvc{hh}", tag=f"pvc{hh}")
                nc.vector.tensor_copy(pvc[:], pvs[hh][:])
                pvcs.append(pvc)
            return pvcs

        def pv_norm(pvcs, p, blk):
            tq0 = blk * 512
            for hh in range(2):
                nc.vector.tensor_copy(zt[hh * 32:hh * 32 + 1, :], pvcs[hh][64:65, :])
            rc = spool.tile([33, 512], F32, name="rc", tag="rc")
            nc.vector.reciprocal(rc[:], zt[:])
            # partition_broadcast always reads the tile's partition 0, so
            # stage head 1's row into a base-0 tile first
            rc1 = spool.tile([1, 512], F32, name="rc1", tag="rc1")
            nc.vector.tensor_copy(rc1[:], rc[32:33, :])
            rcaps = [rc[0:1, :], rc1[:]]
            rbs = []
            for hh in range(2):
                rb = spool.tile([64, 512], F32, name=f"rb{hh}", tag=f"rb{hh}")
                nc.gpsimd.partition_broadcast(rb[:], rcaps[hh])
                rbs.append(rb)
            for hh in range(2):
                nc.vector.tensor_tensor(
                    attnT[p][hh * 64:(hh + 1) * 64, tq0:tq0 + 512],
                    pvcs[hh][0:64, :],
                    rbs[hh][:],
                    mybir.AluOpType.mult,
                )

        def wo_group(tt):
            for eb in range(2):
                ps = ps_b.tile([128, 512], F32, name="pswo", tag="psb")
                for fc in range(2):
                    nc.tensor.matmul(
                        ps[:],
                        attnT[fc][:, tt * 128:(tt + 1) * 128],
                        wo_t[fc][:, eb * 512:(eb + 1) * 512],
                        start=(fc == 0),
                        stop=(fc == 1),
                    )
                ot = opool.tile([128, 512], F32, name="ot", tag="ot")
                nc.vector.tensor_copy(ot[:], ps[:])
                nc.sync.dma_start(
                    io["out"][tt * 128:(tt + 1) * 128, eb * 512:(eb + 1) * 512],
                    ot[:],
                )

        def unit_emit(p, blk, fillers, wo_pending, with_v=False):
            """Scores+exp of (p, blk) in 8 groups of [128,1024] PSUM
            (1 tk tile x 2 heads); one 2048-wide exp per group into a
            per-group bf16 tile. PV of THIS unit runs one group behind
            (chunk g-1 after the scores of group g). Fillers (projection
            blocks / Wo groups) are PE work with no ACT dependency,
            emitted between groups."""
            tq0 = blk * 512
            pvs = [ps_pv.tile([65, 512], F32, name=f"pv{hh}", tag=f"pv{hh}")
                   for hh in range(2)]
            etiles = []
            for g in range(8):
                etile = epool.tile([128, 2 * 1024], BF, name="exp", tag="exp")
                etiles.append(etile)
                for j2 in range(2):
                    tk = g * 2 + j2
                    ps = ps_a.tile([128, 1024], F32, name="psa", tag="psa")
                    for hh in range(2):
                        pp = hh * 64
                        nc.tensor.matmul(
                            ps[:, hh * 512:(hh + 1) * 512],
                            kcT[p][pp:pp + 64, tk * 128:(tk + 1) * 128],
                            qcT[p][pp:pp + 64, tq0:tq0 + 512],
                            start=True,
                            stop=True,
                        )
                    nc.scalar.activation(
                        etile[:, j2 * 1024:(j2 + 1) * 1024],
                        ps[:],
                        mybir.ActivationFunctionType.Exp,
                        scale=1.0 / SCALE,
                    )
                if with_v:
                    v_tile(xv, wv_t, bv_bc, 2 * g)
                    v_tile(xv, wv_t, bv_bc, 2 * g + 1)
                if fillers:
                    fillers.pop(0)()
                if wo_pending:
                    wo_group(wo_pending.pop(0))
                if g >= 1:
                    for j2 in range(2):
                        pv_chunk(pvs, p, etiles[g - 1][:, j2 * 1024:(j2 + 1) * 1024],
                                 2 * (g - 1) + j2)
            for j2 in range(2):
                pv_chunk(pvs, p, etiles[7][:, j2 * 1024:(j2 + 1) * 1024], 14 + j2)
            return pv_copy(pvs)

        # ================= emission schedule =================
        wk_t = load_w("wkT")
        bk_t = load_b("bk")
        xk = load_x("xkT")
        wq_t = load_w("wqT")
        bq_t = load_b("bq")
        xq = load_x("xqT")
        # lead-in: everything unit (0,0) needs
        proj_qk(wk_t, bk_t, kcT, xk, 0)
        proj_qk(wq_t, bq_t, qcT, xq, 0, tbps=(0,))

        wv_t = load_w("wvT")
        xv = load_x("xvT")
        bv_row = const.tile([1, F], F32, name="bvrow", tag="bvrow")
        nc.sync.dma_start(bv_row[:], io["bv"][:])
        bv_full = const.tile([128, F], F32, name="bvbc", tag="bvbc")
        nc.gpsimd.partition_broadcast(bv_full[:], bv_row[:])
        bv_bc = bv_full[:].rearrange("p (h d) -> p h d", h=HPC)
        wo_t = []
        for fc in range(2):
            t = const.tile([128, E], BF, name=f"woT{fc}", tag=f"woT{fc}")
            nc.sync.dma_start(t[:], io["woT"][fc * 128:(fc + 1) * 128, :])
            wo_t.append(t)
        zt = const.tile([33, 512], F32, name="zt", tag="zt")
        nc.vector.memset(zt[:], 1.0)

        # remaining projection work, consumed one block per score group
        fillers = [
            lambda: proj_qk(wq_t, bq_t, qcT, xq, 0, tbps=(1,)),
            lambda: proj_qk(wq_t, bq_t, qcT, xq, 0, tbps=(2,)),
            lambda: proj_qk(wq_t, bq_t, qcT, xq, 0, tbps=(3,)),
            lambda: proj_qk(wq_t, bq_t, qcT, xq, 1, tbps=(0,)),
            lambda: proj_qk(wq_t, bq_t, qcT, xq, 1, tbps=(1,)),
            lambda: proj_qk(wq_t, bq_t, qcT, xq, 1, tbps=(2,)),
            lambda: proj_qk(wq_t, bq_t, qcT, xq, 1, tbps=(3,)),
            lambda: proj_qk(wk_t, bk_t, kcT, xk, 1, tbps=(0,)),
            lambda: proj_qk(wk_t, bk_t, kcT, xk, 1, tbps=(1,)),
            lambda: proj_qk(wk_t, bk_t, kcT, xk, 1, tbps=(2,)),
            lambda: proj_qk(wk_t, bk_t, kcT, xk, 1, tbps=(3,)),
        ]

        wo_pending = []
        units = [(0, 0), (0, 1), (0, 2), (0, 3), (1, 0), (1, 1), (1, 2), (1, 3)]
        for ui, (p, blk) in enumerate(units):
            pvcs = unit_emit(p, blk, fillers, wo_pending, with_v=(ui == 0))
            if io["debug"]:
                for hh in range(2):
                    nc.sync.dma_start(
                        io["d_pvc"][(ui * 2 + hh) * 65:(ui * 2 + hh + 1) * 65, :],
                        pvcs[hh][:],
                    )
            pv_norm(pvcs, p, blk)
            if p == 1:
                wo_pending.extend(range(blk * 4, blk * 4 + 4))
        for f in fillers:
            f()
        for tt in wo_pending:
            wo_group(tt)

        if io["debug"]:
            for ft in range(2):
                nc.sync.dma_start(io["d_qcT"][ft * 128:(ft + 1) * 128, :], qcT[ft][:])
                nc.sync.dma_start(io["d_kcT"][ft * 128:(ft + 1) * 128, :], kcT[ft][:])
                nc.sync.dma_start(io["d_attnT"][ft * 128:(ft + 1) * 128, :], attnT[ft][:])
            for tt in range(16):
                nc.sync.dma_start(
                    io["d_vaug"][tt * 128:(tt + 1) * 128, :],
                    v_pl[tt][:].rearrange("p h d -> p (h d)"),
                )


def _build():
    nc = bacc.Bacc("TRN2", target_bir_lowering=False, debug=False)
    io = {}
    for name, shape, dt in (
        ("xqT", [E, T], BF),
        ("xkT", [E, T], BF),
        ("xvT", [E, T], BF),
        ("wqT", [E, F], BF),
        ("wkT", [E, F], BF),
        ("wvT", [E, F], BF),
        ("woT", [F, E], BF),
        ("bq", [F, 1], F32),
        ("bk", [F, 1], F32),
        ("bv", [1, F], F32),
    ):
        io[name] = nc.dram_tensor(name, shape, dt, kind="ExternalInput").ap()
    io["out"] = nc.dram_tensor("out", [T, E], F32, kind="ExternalOutput").ap()
    import os
    debug = bool(int(os.environ.get("KERNEL_DEBUG", "0")))
    if debug:
        for nm, shape in (("d_qcT", [2 * 128, T]), ("d_kcT", [2 * 128, T]),
                          ("d_attnT", [2 * 128, T]), ("d_vaug", [16 * 128, HPC * (Dh + 1)])):
            io[nm] = nc.dram_tensor(nm, shape, BF, kind="ExternalOutput").ap()
        io["d_pvc"] = nc.dram_tensor("d_pvc", [16 * 65, 512], F32,
                                     kind="ExternalOutput").ap()
    io["debug"] = debug
    with tile.TileContext(nc) as tc:
        _emit(tc, io)
    nc.compile()
    return nc


def _fold_clr(W, b, clr):
    """q_c = q - mean_head(q) + clr  ==  x @ (C W).T + (C b + clr)."""
    W64 = W.astype(np.float64).reshape(H, Dh, E)
    W_eff = W64 - W64.mean(axis=1, keepdims=True)
    b64 = b.astype(np.float64).reshape(H, Dh)
    b_eff = b64 - b64.mean(axis=1, keepdims=True) + clr.astype(np.float64).reshape(H, Dh)
    return W_eff.reshape(E, E), b_eff.reshape(E)


_NC_CACHE = None


def kernel(**inputs):
    global _NC_CACHE
    query = np.asarray(inputs["query"], np.float32)
    key = np.asarray(inputs["key"], np.float32)
    value = np.asarray(inputs["value"], np.float32)
    mask = np.asarray(inputs["key_padding_mask"])
    Wq, bq = np.asarray(inputs["Wq"], np.float32), np.asarray(inputs["bq"], np.float32)
    Wk, bk = np.asarray(inputs["Wk"], np.float32), np.asarray(inputs["bk"], np.float32)
    Wv, bv = np.asarray(inputs["Wv"], np.float32), np.asarray(inputs["bv"], np.float32)
    Wo, bo = np.asarray(inputs["Wo"], np.float32), np.asarray(inputs["bo"], np.float32)
    cq = np.asarray(inputs["clr_bias_q"], np.float32)
    ck = np.asarray(inputs["clr_bias_k"], np.float32)
    assert not mask.any(), "kernel assumes empty key_padding_mask"

    Wq_eff, bq_eff = _fold_clr(Wq, bq, cq)
    Wk_eff, bk_eff = _fold_clr(Wk, bk, ck)

    def bf(x):
        return np.ascontiguousarray(x.astype(np.float32)).astype(BF_NP)

    in_maps = []
    for c in range(NCORES):
        b = c // 4
        fs = (c % 4) * F
        m = {
            "xqT": bf(query[b].T),
            "xkT": bf(key[b].T),
            "xvT": bf(value[b].T),
            "wqT": bf(Wq_eff[fs:fs + F].T),
            "wkT": bf(Wk_eff[fs:fs + F].T),
            "wvT": bf(Wv[fs:fs + F].T),
            "woT": bf(Wo[:, fs:fs + F].T),
            "bq": np.ascontiguousarray(bq_eff[fs:fs + F, None], dtype=np.float32),
            "bk": np.ascontiguousarray(bk_eff[fs:fs + F, None], dtype=np.float32),
            "bv": np.ascontiguousarray(bv[None, fs:fs + F], dtype=np.float32),
        }
        in_maps.append(m)

    if _NC_CACHE is None:
        _NC_CACHE = _build()
    nc = _NC_CACHE

    import os

    trace = bool(int(os.environ.get("KERNEL_TRACE", "0")))
    if trace:
        _install_ntff_hook()
    res = None
    last_exc = None
    out = None
    for attempt in range(4):
        try:
            res = run_bass_kernel_spmd(
                nc, in_maps, core_ids=list(range(NCORES)), trace=trace
            )
        except Exception as e:  # transient NRT_EXEC_UNIT_UNRECOVERABLE etc.
            last_exc = e
            import time

            time.sleep(2.0)
            continue
        out = np.zeros((B, T, E), np.float32)
        for c in range(NCORES):
            out[c // 4] += res.results[c]["out"]
        if np.isfinite(out).all():
            break
        out = None  # rare transient corruption: retry
    if out is None:
        if last_exc is not None and res is None:
            raise last_exc
        raise RuntimeError("kernel produced non-finite output on all attempts")
    kernel.last_results = res
    out += bo[None, None, :].astype(np.float32)
    return out


# revision 17
# speedup vs baseline: 1.0618x; 1.0618x over previous
"""Aitchison multi-head attention on 8 trn2 NeuronCores.

Strategy:
- CLR centering is linear -> folded into Wq/Wk + biases on the host (fp64).
- Shard: core c handles batch b=c//4 and 4 heads (feature slice of 256).
  QKV/out projection weights sliced per core; host sums the 4 partial
  output projections per batch and adds bo.
- Device kernel (per core, bf16 matmul operands / fp32 PSUM accum):
    qcT,kcT [256,2048] = W_eff @ x.T (+bias, f on partitions)
    v_pl    [2048, 4x65] = x @ Wv.T + bv with a ones column appended per
            head: the PV matmul (lhsT=[tk,65]) then produces the softmax
            denominator as PSUM row 64 for free -- no separate ones-lhs
            sums matmuls (those were 25% of all PE streaming cycles).
    Per unit (head-pair p, 512-wide q block): 8 score groups, each
    [128,1024] PSUM (1 tk tile x 2 heads) x2 double-buffered; one
    2048-wide exp per group into a per-group [128,2048] bf16 tile
    (bufs=4 rolling window keeps SBUF small).
    PV runs IN-unit, one group behind the exp (chunk g-1 after scores of
    group g), into two per-head [65,512] PSUM banks; banks are copied to
    SBUF right after chunk 7 so the next unit's start=True chain reuses
    them after a ~1-group handoff.
    1/Z via reciprocal_approx_fast (5x plain reciprocal), broadcast on
    GPSIMD, normalize mult on DVE into attnT (bf16).
    out partial = attnT.T @ WoT per 128x512 tile -> DVE copy -> DMA out.
- Emission is software-pipelined: the Q/K/V projections beyond the
  lead-in (kcT[0] + first qcT block) are interleaved into early units as
  PE filler; Wo groups drain during the last units and the tail.
"""
import sys
import types

sys.path.insert(0, "/opt/trn_rl_repo")

import numpy as np
import ml_dtypes

import concourse.bass as bass
import concourse.tile as tile
from concourse import bacc, mybir
from concourse.bass_utils import run_bass_kernel_spmd

B, T, E, H, Dh = 2, 2048, 1024, 16, 64
NCORES = 8
HPC = 4            # heads per core
F = HPC * Dh       # 256 features per core
SCALE = 8.0        # sqrt(Dh)
KC = E // 128      # 8 k-chunks in projections
BF = mybir.dt.bfloat16
F32 = mybir.dt.float32
BF_NP = ml_dtypes.bfloat16


def _install_ntff_hook():
    """trace=True under axon needs antenv.axon_hooks, missing in this image."""
    if "antenv.axon_hooks" in sys.modules:
        return
    try:
        from trn_agent_boot.trn_boot import _ntff_profile_via_ctypes

        hook = _ntff_profile_via_ctypes("/opt/axon/libaxon_pjrt.so")
    except Exception:
        hook = None
    mod = types.ModuleType("antenv.axon_hooks")
    mod.get_axon_ntff_profile_hook = lambda: hook
    sys.modules["antenv.axon_hooks"] = mod


def _emit(tc, io):
    nc = tc.nc
    from contextlib import ExitStack

    ctx = ExitStack()
    with ctx:
        const = ctx.enter_context(tc.tile_pool(name="const", bufs=1))
        xpool = ctx.enter_context(tc.tile_pool(name="x", bufs=24))
        qk = ctx.enter_context(tc.tile_pool(name="qk", bufs=1))
        epool = ctx.enter_context(tc.tile_pool(name="exp", bufs=4))
        spool = ctx.enter_context(tc.tile_pool(name="small", bufs=2))
        opool = ctx.enter_context(tc.tile_pool(name="out", bufs=2))
        ps_a = ctx.enter_context(tc.tile_pool(name="psa", bufs=2, space="PSUM"))
        ps_pv = ctx.enter_context(tc.tile_pool(name="pspv", bufs=1, space="PSUM"))
        ps_b = ctx.enter_context(tc.tile_pool(name="psb", bufs=2, space="PSUM"))

        def load_w(name):
            ts = []
            for kk in range(KC):
                t = const.tile([128, F], BF, name=f"{name}{kk}", tag=f"{name}{kk}")
                nc.sync.dma_start(t[:], io[name][kk * 128:(kk + 1) * 128, :])
                ts.append(t)
            return ts

        def load_b(name):
            ts = []
            for ft in range(2):
                t = const.tile([128, 1], F32, name=f"{name}{ft}", tag=f"{name}{ft}")
                nc.sync.dma_start(t[:], io[name][ft * 128:(ft + 1) * 128, :])
                ts.append(t)
            return ts

        def load_x(which):
            xc = []
            for kk in range(KC):
                t = xpool.tile([128, T], BF, name="xc", tag="xc")
                nc.sync.dma_start(t[:], io[which][kk * 128:(kk + 1) * 128, :])
                xc.append(t)
            return xc

        # ---- persistent activation tiles ----
        qcT = [qk.tile([128, T], BF, name=f"qcT{ft}", tag=f"qcT{ft}") for ft in range(2)]
        kcT = [qk.tile([128, T], BF, name=f"kcT{ft}", tag=f"kcT{ft}") for ft in range(2)]
        attnT = [qk.tile([128, T], BF, name=f"attnT{ft}", tag=f"attnT{ft}") for ft in range(2)]
        # v with a ones column per head: [t, 4 heads x (64 v | 1 one)]
        v_pl = [qk.tile([128, HPC, Dh + 1], BF, name=f"vpl{tt}", tag=f"vpl{tt}")
                for tt in range(16)]

        def proj_qk(wt, bt, dst, xc, ft, tbps=(0, 1, 2, 3)):
            for tbp in tbps:  # 512-wide t groups (psb rotation, no stalls)
                ps = ps_b.tile([128, 512], F32, name="psp", tag="psb")
                tq0 = tbp * 512
                for kk in range(KC):
                    nc.tensor.matmul(
                        ps[:],
                        wt[kk][:, ft * 128:(ft + 1) * 128],
                        xc[kk][:, tq0:tq0 + 512],
                        start=(kk == 0),
                        stop=(kk == KC - 1),
                    )
                nc.vector.tensor_scalar_add(
                    dst[ft][:, tq0:tq0 + 512], ps[:], bt[ft][:]
                )

        def v_tile(xc, wv_t, bv_bc, tt):
            ps = ps_b.tile([128, 256], F32, name="psv", tag="psb")
            for kk in range(KC):
                nc.tensor.matmul(
                    ps[:],
                    xc[kk][:, tt * 128:(tt + 1) * 128],
                    wv_t[kk][:],
                    start=(kk == 0),
                    stop=(kk == KC - 1),
                )
            # bias-add into the per-head 64-wide blocks (ones col untouched)
            nc.vector.tensor_tensor(
                v_pl[tt][:, :, 0:Dh],
                ps[:].rearrange("p (h d) -> p h d", h=HPC),
                bv_bc[:, :, :],
                mybir.AluOpType.add,
            )
            nc.gpsimd.memset(v_pl[tt][:, :, Dh:Dh + 1], 1.0)

        # --- PV: per-head [65,512] PSUM banks; row 64 = softmax sum (ones
        # column of v). start=True on the first chunk owns the bank.
        def pv_chunk(pvs, p, etile, g):
            for hh in range(2):
                lh = p * 2 + hh
                sl = etile[:, hh * 512:(hh + 1) * 512]
                nc.tensor.matmul(
                    pvs[hh][:],
                    v_pl[g][:, lh, :],
                    sl,
                    start=(g == 0),
                    stop=(g == 15),
                    skip_group_check=True,
                )

        def pv_finish(pvs, p, blk):
            """Z rows staged straight from PSUM, then the big pv copies
            (releasing the banks), then recip -> broadcast -> normalize."""
            tq0 = blk * 512
            for hh in range(2):
                nc.vector.tensor_copy(zt[hh * 32:hh * 32 + 1, :], pvs[hh][64:65, :])
            pvcs = []
            for hh in range(2):
                pvc = spool.tile([64, 512], F32, name=f"pvc{hh}", tag=f"pvc{hh}")
                nc.vector.tensor_copy(pvc[:], pvs[hh][0:64, :])
                pvcs.append(pvc)
            rc = spool.tile([33, 512], F32, name="rc", tag="rc")
            nc.vector.reciprocal(rc[:], zt[:])
            # partition_broadcast always reads the tile's partition 0, so
            # stage head 1's row into a base-0 tile first
            rc1 = spool.tile([1, 512], F32, name="rc1", tag="rc1")
            nc.vector.tensor_copy(rc1[:], rc[32:33, :])
            rcaps = [rc[0:1, :], rc1[:]]
            rbs = []
            for hh in range(2):
                rb = spool.tile([64, 512], F32, name=f"rb{hh}", tag=f"rb{hh}")
                nc.gpsimd.partition_broadcast(rb[:], rcaps[hh])
                rbs.append(rb)
            for hh in range(2):
                nc.vector.tensor_tensor(
                    attnT[p][hh * 64:(hh + 1) * 64, tq0:tq0 + 512],
                    pvcs[hh][:],
                    rbs[hh][:],
                    mybir.AluOpType.mult,
                )
            return pvcs

        def wo_group(tt):
            for eb in range(2):
                ps = ps_b.tile([128, 512], F32, name="pswo", tag="psb")
                for fc in range(2):
                    nc.tensor.matmul(
                        ps[:],
                        attnT[fc][:, tt * 128:(tt + 1) * 128],
                        wo_t[fc][:, eb * 512:(eb + 1) * 512],
                        start=(fc == 0),
                        stop=(fc == 1),
                    )
                ot = opool.tile([128, 512], F32, name="ot", tag="ot")
                nc.vector.tensor_copy(ot[:], ps[:])
                nc.sync.dma_start(
                    io["out"][tt * 128:(tt + 1) * 128, eb * 512:(eb + 1) * 512],
                    ot[:],
                )

        def unit_emit(p, blk, fillers, wo_pending):
            """Scores+exp of (p, blk) in 8 groups of [128,1024] PSUM
            (1 tk tile x 2 heads); one 2048-wide exp per group into a
            per-group bf16 tile. PV of THIS unit runs one group behind
            (chunk g-1 after the scores of group g). Fillers (projection
            blocks) are PE work with no ACT dependency; Wo groups are
            popped only at g>=4 so the previous unit's normalize chain
            (recip/broadcast on DVE+GPSIMD) has slack and never stalls
            the PE (a >3.4us PE stall makes the HAM re-throttle the
            clock, costing ~10us each time)."""
            tq0 = blk * 512
            pvs = [ps_pv.tile([65, 512], F32, name=f"pv{hh}", tag=f"pv{hh}")
                   for hh in range(2)]
            etiles = []
            for g in range(8):
                etile = epool.tile([128, 2 * 1024], BF, name="exp", tag="exp")
                etiles.append(etile)
                for j2 in range(2):
                    tk = g * 2 + j2
                    ps = ps_a.tile([128, 1024], F32, name="psa", tag="psa")
                    for hh in range(2):
                        pp = hh * 64
                        nc.tensor.matmul(
                            ps[:, hh * 512:(hh + 1) * 512],
                            kcT[p][pp:pp + 64, tk * 128:(tk + 1) * 128],
                            qcT[p][pp:pp + 64, tq0:tq0 + 512],
                            start=True,
                            stop=True,
                        )
                    nc.scalar.activation(
                        etile[:, j2 * 1024:(j2 + 1) * 1024],
                        ps[:],
                        mybir.ActivationFunctionType.Exp,
                        scale=1.0 / SCALE,
                    )
                if fillers:
                    fillers.pop(0)()
                if g >= 4 and wo_pending:
                    wo_group(wo_pending.pop(0))
                if g >= 1:
                    for j2 in range(2):
                        pv_chunk(pvs, p, etiles[g - 1][:, j2 * 1024:(j2 + 1) * 1024],
                                 2 * (g - 1) + j2)
            for j2 in range(2):
                pv_chunk(pvs, p, etiles[7][:, j2 * 1024:(j2 + 1) * 1024], 14 + j2)
            return pvs

        # ================= emission schedule =================
        # DMA queue order = the order tensors are first needed; the lead-in
        # keeps the PE busy on work gated only by already-arrived tensors
        # while the rest of the 12MB of activations stream in.
        wk_t = load_w("wkT")
        bk_t = load_b("bk")
        xk = load_x("xkT")
        wv_t = load_w("wvT")
        bv_row = const.tile([1, F], F32, name="bvrow", tag="bvrow")
        nc.sync.dma_start(bv_row[:], io["bv"][:])
        xv = load_x("xvT")
        wq_t = load_w("wqT")
        bq_t = load_b("bq")
        xq = load_x("xqT")
        wo_t = []
        for fc in range(2):
            t = const.tile([128, E], BF, name=f"woT{fc}", tag=f"woT{fc}")
            nc.sync.dma_start(t[:], io["woT"][fc * 128:(fc + 1) * 128, :])
            wo_t.append(t)

        bv_full = const.tile([128, F], F32, name="bvbc", tag="bvbc")
        nc.gpsimd.partition_broadcast(bv_full[:], bv_row[:])
        bv_bc = bv_full[:].rearrange("p (h d) -> p h d", h=HPC)
        zt = const.tile([33, 512], F32, name="zt", tag="zt")
        nc.vector.memset(zt[:], 1.0)

        # lead-in PE work, each stage gated only on tensors already loaded:
        proj_qk(wk_t, bk_t, kcT, xk, 0)          # needs xk
        proj_qk(wk_t, bk_t, kcT, xk, 1)          # needs xk
        for tt in range(16):                      # needs xv
            v_tile(xv, wv_t, bv_bc, tt)
        proj_qk(wq_t, bq_t, qcT, xq, 0, tbps=(0,))  # needs xq

        # remaining q projection, consumed one block per score group
        fillers = [
            lambda: proj_qk(wq_t, bq_t, qcT, xq, 0, tbps=(1,)),
            lambda: proj_qk(wq_t, bq_t, qcT, xq, 0, tbps=(2,)),
            lambda: proj_qk(wq_t, bq_t, qcT, xq, 0, tbps=(3,)),
            lambda: proj_qk(wq_t, bq_t, qcT, xq, 1, tbps=(0,)),
            lambda: proj_qk(wq_t, bq_t, qcT, xq, 1, tbps=(1,)),
            lambda: proj_qk(wq_t, bq_t, qcT, xq, 1, tbps=(2,)),
            lambda: proj_qk(wq_t, bq_t, qcT, xq, 1, tbps=(3,)),
        ]

        def warm(n):
            # dummy LDWEIGHTS: keep the PE activity monitor from
            # re-throttling the clock during unavoidable PE slack
            for _ in range(n):
                nc.tensor.ldweights(wo_t[0][:, 0:128])

        wo_pending = []
        units = [(0, 0), (0, 1), (0, 2), (0, 3), (1, 0), (1, 1), (1, 2), (1, 3)]
        for ui, (p, blk) in enumerate(units):
            pvs = unit_emit(p, blk, fillers, wo_pending)
            pvcs = pv_finish(pvs, p, blk)
            if io["debug"]:
                for hh in range(2):
                    nc.sync.dma_start(
                        io["d_pvc"][(ui * 2 + hh) * 65:(ui * 2 + hh + 1) * 65, 0:1],
                        zt[hh * 32:hh * 32 + 1, 0:1],
                    )
                    nc.sync.dma_start(
                        io["d_pvc"][(ui * 2 + hh) * 65 + 1:(ui * 2 + hh) * 65 + 65, :],
                        pvcs[hh][:],
                    )
            if p == 1:
                wo_pending.extend(range(blk * 4, blk * 4 + 4))
        # tail: blk2's Wo is ready (normalized during unit 7); blk3 waits on
        # the last normalize chain -- keep the PE warm across that wait.
        ready = [tt for tt in wo_pending if tt < 12]
        late = [tt for tt in wo_pending if tt >= 12]
        for tt in ready:
            wo_group(tt)
        warm(36)
        for tt in late:
            wo_group(tt)

        if io["debug"]:
            for ft in range(2):
                nc.sync.dma_start(io["d_qcT"][ft * 128:(ft + 1) * 128, :], qcT[ft][:])
                nc.sync.dma_start(io["d_kcT"][ft * 128:(ft + 1) * 128, :], kcT[ft][:])
                nc.sync.dma_start(io["d_attnT"][ft * 128:(ft + 1) * 128, :], attnT[ft][:])
            for tt in range(16):
                nc.sync.dma_start(
                    io["d_vaug"][tt * 128:(tt + 1) * 128, :],
                    v_pl[tt][:].rearrange("p h d -> p (h d)"),
                )


def _build():
    nc = bacc.Bacc("TRN2", target_bir_lowering=False, debug=False)
    io = {}
    for name, shape, dt in (
        ("xqT", [E, T], BF),
        ("xkT", [E, T], BF),
        ("xvT", [E, T], BF),
        ("wqT", [E, F], BF),
        ("wkT", [E, F], BF),
        ("wvT", [E, F], BF),
        ("woT", [F, E], BF),
        ("bq", [F, 1], F32),
        ("bk", [F, 1], F32),
        ("bv", [1, F], F32),
    ):
        io[name] = nc.dram_tensor(name, shape, dt, kind="ExternalInput").ap()
    io["out"] = nc.dram_tensor("out", [T, E], F32, kind="ExternalOutput").ap()
    import os
    debug = bool(int(os.environ.get("KERNEL_DEBUG", "0")))
    if debug:
        for nm, shape in (("d_qcT", [2 * 128, T]), ("d_kcT", [2 * 128, T]),
                          ("d_attnT", [2 * 128, T]), ("d_vaug", [16 * 128, HPC * (Dh + 1)])):
            io[nm] = nc.dram_tensor(nm, shape, BF, kind="ExternalOutput").ap()
        io["d_pvc"] = nc.dram_tensor("d_pvc", [16 * 65, 512], F32,
                                     kind="ExternalOutput").ap()
    io["debug"] = debug
    with tile.TileContext(nc) as tc:
        _emit(tc, io)
    nc.compile()
    return nc


def _fold_clr(W, b, clr):
    """q_c = q - mean_head(q) + clr  ==  x @ (C W).T + (C b + clr)."""
    W64 = W.astype(np.float64).reshape(H, Dh, E)
    W_eff = W64 - W64.mean(axis=1, keepdims=True)
    b64 = b.astype(np.float64).reshape(H, Dh)
    b_eff = b64 - b64.mean(axis=1, keepdims=True) + clr.astype(np.float64).reshape(H, Dh)
    return W_eff.reshape(E, E), b_eff.reshape(E)


_NC_CACHE = None


def kernel(**inputs):
    global _NC_CACHE
    query = np.asarray(inputs["query"], np.float32)
    key = np.asarray(inputs["key"], np.float32)
    value = np.asarray(inputs["value"], np.float32)
    mask = np.asarray(inputs["key_padding_mask"])
    Wq, bq = np.asarray(inputs["Wq"], np.float32), np.asarray(inputs["bq"], np.float32)
    Wk, bk = np.asarray(inputs["Wk"], np.float32), np.asarray(inputs["bk"], np.float32)
    Wv, bv = np.asarray(inputs["Wv"], np.float32), np.asarray(inputs["bv"], np.float32)
    Wo, bo = np.asarray(inputs["Wo"], np.float32), np.asarray(inputs["bo"], np.float32)
    cq = np.asarray(inputs["clr_bias_q"], np.float32)
    ck = np.asarray(inputs["clr_bias_k"], np.float32)
    assert not mask.any(), "kernel assumes empty key_padding_mask"

    Wq_eff, bq_eff = _fold_clr(Wq, bq, cq)
    Wk_eff, bk_eff = _fold_clr(Wk, bk, ck)

    def bf(x):
        return np.ascontiguousarray(x.astype(np.float32)).astype(BF_NP)

    in_maps = []
    for c in range(NCORES):
        b = c // 4
        fs = (c % 4) * F
        m = {
            "xqT": bf(query[b].T),
            "xkT": bf(key[b].T),
            "xvT": bf(value[b].T),
            "wqT": bf(Wq_eff[fs:fs + F].T),
            "wkT": bf(Wk_eff[fs:fs + F].T),
            "wvT": bf(Wv[fs:fs + F].T),
            "woT": bf(Wo[:, fs:fs + F].T),
            "bq": np.ascontiguousarray(bq_eff[fs:fs + F, None], dtype=np.float32),
            "bk": np.ascontiguousarray(bk_eff[fs:fs + F, None], dtype=np.float32),
            "bv": np.ascontiguousarray(bv[None, fs:fs + F], dtype=np.float32),
        }
        in_maps.append(m)

    if _NC_CACHE is None:
        _NC_CACHE = _build()
    nc = _NC_CACHE

    import os

    trace = bool(int(os.environ.get("KERNEL_TRACE", "0")))
    if trace:
        _install_ntff_hook()
    res = None
    last_exc = None
    out = None
    for attempt in range(4):
        try:
            res = run_bass_kernel_spmd(
                nc, in_maps, core_ids=list(range(NCORES)), trace=trace
            )
        except Exception as e:  # transient NRT_EXEC_UNIT_UNRECOVERABLE etc.
            last_exc = e
            import time

            time.sleep(2.0)
            continue
        out = np.zeros((B, T, E), np.float32)
        for c in range(NCORES):
            out[c // 4] += res.results[c]["out"]
        if np.isfinite(out).all():
            break
        out = None  # rare transient corruption: retry
    if out is None:
        if last_exc is not None and res is None:
            raise last_exc
        raise RuntimeError("kernel produced non-finite output on all attempts")
    kernel.last_results = res
    out += bo[None, None, :].astype(np.float32)
    return out
